# revision 2
# baseline (speedup 1.0000x reference)
"""Trainium2 kernel for: out = tanh(x @ scatter_nd(nonzero_ind, kernel_vector, (20000, 4096)) + bias).

Strategy (8 NeuronCores):
  - Host builds the dense (20000, 4096) fp16 weight matrix from the COO
    triples, pads K to 157*128 = 20096, and pre-transposes x to fp16
    xT (20096, 2048), shared by all cores.
  - Shard units 8-ways: core c owns output columns [c*512, (c+1)*512).
    No K-split => 157 K-tiles (vs 160 when K is quartered and padded),
    and no host-side partial summation.
  - On device: W panel (157 x [128 x 512] fp16, 157 KB/partition) is
    SBUF-resident, streamed just-in-time during the first batch pass.
    Batch is processed in 4 passes of 512 columns; each pass holds
    4 PSUM banks (one per 128-unit chunk), ping-ponging between two
    bank sets so pass epilogues are off the critical path. Stationary
    operand = W chunk [128k x 128u], moving = xT tile [128k x 512b].
  - PE warmup: dummy matmuls on a memset tile cover the initial DMA
    fill so the PE p-state ramp (0.65/1.2 GHz -> 2.4 GHz after ~3us)
    completes before real work starts.
  - Epilogue fused on the Activation engine: out = tanh(psum + bias)
    straight from PSUM, per-partition bias, fp16 output. Host only
    concatenates/transposes the 8 shards.
"""

import numpy as np

P = 128
B, K, U = 2048, 20000, 4096
KT = 157                 # ceil(20000 / 128)
KPAD = KT * P            # 20096
U_SH = U // 8            # 512 units per core
UC = U_SH // P           # 4 unit chunks per core
QB = 512                 # batch quarter width
NQ = B // QB             # 4 batch passes
N_WARM = 44              # warmup matmuls (free dim 256) covering DMA fill
WARM_F = 256

TRACE = False            # set by test harness for profiled runs
LAST_RESULT = None       # BassKernelResults of the last run (for the harness)

_NC_CACHE = {}


def _build_nc():
    from concourse import bacc
    import concourse.mybir as mybir
    import concourse.tile as tile

    f32 = mybir.dt.float32
    f16 = mybir.dt.float16
    Tanh = mybir.ActivationFunctionType.Tanh

    nc = bacc.Bacc("TRN2", target_bir_lowering=False, debug=False)
    xt_d = nc.dram_tensor("xt_sh", [KPAD, B], f16, kind="ExternalInput").ap()
    w_d = nc.dram_tensor("w_sh", [KPAD, U_SH], f16, kind="ExternalInput").ap()
    b_d = nc.dram_tensor("b_sh", [UC, P, 1], f32, kind="ExternalInput").ap()
    o_d = nc.dram_tensor("out_p", [U_SH, B], f16, kind="ExternalOutput").ap()

    with tile.TileContext(nc) as tc:
        with (
            tc.tile_pool(name="wpanel", bufs=1) as wpool,
            tc.tile_pool(name="xstream", bufs=6) as xpool,
            tc.tile_pool(name="bias", bufs=1) as bpool,
            tc.tile_pool(name="ostage", bufs=4) as opool,
            tc.tile_pool(name="warm", bufs=1) as warmpool,
            tc.tile_pool(name="mpsum", bufs=1, space="PSUM") as mpsum,
        ):
            # PE warmup: ramp the tensor-engine p-state while the first
            # real tiles stream in. Garbage-free via memset; results land
            # in a PSUM bank that the first real accumulation overwrites
            # (start=True resets the bank).
            warm = warmpool.tile([P, WARM_F], f16, tag="warm", name="warm")
            nc.vector.memset(warm[:], 0.0)
            warm_ps = mpsum.tile([P, WARM_F], f32, tag="ps0", name="warm_ps")
            for _ in range(N_WARM):
                nc.tensor.matmul(
                    warm_ps[:], warm[:, :P], warm[:], start=True, stop=True
                )

            # Per-partition bias chunks (psum partition dim = units).
            bias_t = []
            for uc in range(UC):
                bt = bpool.tile([P, 1], f32, tag=f"b{uc}", name=f"b{uc}")
                nc.sync.dma_start(bt[:], b_d[uc])
                bias_t.append(bt)

            # Resident W panel, 157 x [128 x 512] fp16. DMAs are issued
            # up front; the tile framework paces them, and the first batch
            # pass consumes them just-in-time.
            wt = [
                wpool.tile([P, U_SH], f16, tag=f"w{kt}", name=f"w{kt}")
                for kt in range(KT)
            ]

            for q in range(NQ):
                psums = [
                    mpsum.tile(
                        [P, QB], f32,
                        tag=f"ps{4 * (q % 2) + uc}",
                        name=f"ps{q}_{uc}",
                    )
                    for uc in range(UC)
                ]
                for kt in range(KT):
                    if q == 0:
                        nc.sync.dma_start(wt[kt][:], w_d[kt * P:(kt + 1) * P, :])
                    xt = xpool.tile([P, QB], f16, tag="xt", name=f"x{q}_{kt}")
                    nc.sync.dma_start(
                        xt[:], xt_d[kt * P:(kt + 1) * P, q * QB:(q + 1) * QB]
                    )
                    for uc in range(UC):
                        nc.tensor.matmul(
                            psums[uc][:],
                            wt[kt][:, uc * P:(uc + 1) * P],
                            xt[:],
                            start=(kt == 0),
                            stop=(kt == KT - 1),
                        )
                # Fused epilogue: tanh(psum + bias) -> fp16, straight from
                # PSUM on the Activation engine; overlaps the next pass.
                for uc in range(UC):
                    ot = opool.tile([P, QB], f16, tag="ot", name=f"o{q}_{uc}")
                    nc.scalar.activation(
                        ot[:], psums[uc][:], Tanh, bias=bias_t[uc][:], scale=1.0
                    )
                    nc.sync.dma_start(
                        o_d[uc * P:(uc + 1) * P, q * QB:(q + 1) * QB], ot[:]
                    )

    nc.compile()
    return nc


def _get_nc(key=("v2",)):
    if key not in _NC_CACHE:
        _NC_CACHE[key] = _build_nc()
    return _NC_CACHE[key]


def kernel(x, kernel_vector, bias, nonzero_ind):
    global LAST_RESULT
    from concourse.bass_utils import run_bass_kernel_spmd

    x = np.asarray(x, dtype=np.float32)
    kernel_vector = np.asarray(kernel_vector, dtype=np.float32)
    bias = np.asarray(bias, dtype=np.float32)
    nonzero_ind = np.asarray(nonzero_ind)

    nc = _get_nc()

    # Host scatter: dense fp16 weights, K padded to 157*128.
    rows = nonzero_ind[:, 0].astype(np.int64)
    cols = nonzero_ind[:, 1].astype(np.int64)
    w_full = np.zeros(KPAD * U, np.float32)
    np.add.at(w_full, rows * U + cols, kernel_vector)
    w_full = w_full.reshape(KPAD, U).astype(np.float16)

    # Shared transposed x, fp16, K-padded.
    xt = np.zeros((KPAD, B), np.float16)
    xt[:K] = x.astype(np.float16).T

    in_maps = []
    for c in range(8):
        in_maps.append({
            "xt_sh": xt,
            "w_sh": np.ascontiguousarray(w_full[:, c * U_SH:(c + 1) * U_SH]),
            "b_sh": np.ascontiguousarray(
                bias[c * U_SH:(c + 1) * U_SH].reshape(UC, P, 1)
            ),
        })

    kwargs = {}
    if TRACE:
        kwargs = dict(trace=True, trace_cores=list(range(8)))
    res = run_bass_kernel_spmd(nc, in_maps, core_ids=list(range(8)), **kwargs)
    LAST_RESULT = res

    # Device already applied bias + tanh; just assemble (out is [U, B]).
    out_t = np.concatenate(
        [res.results[c]["out_p"] for c in range(8)], axis=0
    )
    return out_t.T.astype(np.float32)


# revision 5
# speedup vs baseline: 1.0322x; 1.0322x over previous
"""Trainium2 kernel for: out = tanh(x @ scatter_nd(nonzero_ind, kernel_vector, (20000, 4096)) + bias).

Strategy (8 NeuronCores):
  - Host builds the dense (20000, 4096) fp16 weight matrix from the COO
    triples, pads K to 157*128 = 20096, and pre-transposes x to fp16
    xT (20096, 2048), shared by all cores.
  - Shard units 8-ways: core c owns output columns [c*512, (c+1)*512).
    No K-split => 157 K-tiles (vs 160 for a padded K-quartering), and no
    host-side partial summation.
  - Output-stationary windowed accumulation: K is processed in 8 growing
    windows [4,6,10,16,26,40,40,15]. For each window, 4 batch passes of
    512 columns run over the window's K-tiles, accumulating in 4 PSUM
    banks (ping-ponged between window parities); window partials are
    folded into SBUF fp32 accumulators on the DVE. Growing windows let
    the W panel stream in at a flat ~60 GB/s alongside the 150 GB/s x
    stream instead of demanding 300+ GB/s up front (which starves the
    PE: HBM tops out ~230-290 GB/s effective).
  - The last window preloads ACC back into PSUM (DVE copy) and matmuls
    continue with start=False, so the final epilogue still reads PSUM:
    fused out = tanh(psum + bias) on the Activation engine, fp16 out.
  - PE warmup: dummy matmuls on a memset tile cover the initial DMA fill
    so the PE p-state ramp (-> 2.4 GHz after ~3us) completes before real
    work starts.
  - Host only concatenates/transposes the 8 shards.
"""

import numpy as np

P = 128
B, K, U = 2048, 20000, 4096
KT = 157                 # ceil(20000 / 128)
KPAD = KT * P            # 20096
U_SH = U // 8            # 512 units per core
UC = U_SH // P           # 4 unit chunks per core
QB = 512                 # batch quarter width
NQ = B // QB             # 4 batch passes
WINS = [4, 6, 10, 16, 26, 40, 40, 15]   # K-window sizes (sum = 157)
N_WARM = 44              # warmup matmuls (free dim 256) covering DMA fill
WARM_F = 256

TRACE = False            # set by test harness for profiled runs
LAST_RESULT = None       # BassKernelResults of the last run (for the harness)

_NC_CACHE = {}


def _build_nc():
    from concourse import bacc
    import concourse.mybir as mybir
    import concourse.tile as tile

    f32 = mybir.dt.float32
    f16 = mybir.dt.float16
    Tanh = mybir.ActivationFunctionType.Tanh
    add_op = mybir.AluOpType.add

    assert sum(WINS) == KT
    NW = len(WINS)
    offs = [sum(WINS[:i]) for i in range(NW)]
    max_w = max(WINS)

    nc = bacc.Bacc("TRN2", target_bir_lowering=False, debug=False)
    xt_d = nc.dram_tensor("xt_sh", [KPAD, B], f16, kind="ExternalInput").ap()
    w_d = nc.dram_tensor("w_sh", [KPAD, U_SH], f16, kind="ExternalInput").ap()
    b_d = nc.dram_tensor("b_sh", [UC, P, 1], f32, kind="ExternalInput").ap()
    o_d = nc.dram_tensor("out_p", [U_SH, B], f16, kind="ExternalOutput").ap()

    with tile.TileContext(nc) as tc:
        with (
            tc.tile_pool(name="wpanel", bufs=3) as wpool,
            tc.tile_pool(name="xstream", bufs=16) as xpool,
            tc.tile_pool(name="accpool", bufs=1) as apool,
            tc.tile_pool(name="bias", bufs=1) as bpool,
            tc.tile_pool(name="ostage", bufs=4) as opool,
            tc.tile_pool(name="warm", bufs=1) as warmpool,
            tc.tile_pool(name="mpsum", bufs=1, space="PSUM") as mpsum,
        ):
            # PE warmup: ramp the tensor-engine p-state while the first
            # real tiles stream in. Results land in a PSUM bank that the
            # first real accumulation overwrites (start=True resets it).
            warm = warmpool.tile([P, WARM_F], f16, tag="warm", name="warm")
            nc.vector.memset(warm[:], 0.0)
            warm_ps = mpsum.tile([P, WARM_F], f32, tag="ps0", name="warm_ps")
            for _ in range(N_WARM):
                nc.tensor.matmul(
                    warm_ps[:], warm[:, :P], warm[:], start=True, stop=True
                )

            # Per-partition bias chunks (psum partition dim = units).
            bias_t = []
            for uc in range(UC):
                bt = bpool.tile([P, 1], f32, tag=f"b{uc}", name=f"b{uc}")
                nc.sync.dma_start(bt[:], b_d[uc])
                bias_t.append(bt)

            # W window-slot tiles: tag per slot, 3 deep so the next
            # window's prefetch never waits on the current window.
            def w_tile(j, wi):
                return wpool.tile([P, U_SH], f16, tag=f"w{j}", name=f"w{wi}_{j}")

            def w_dma(wi, j):
                kt = offs[wi] + j
                t = w_tile(j, wi)
                nc.sync.dma_start(t[:], w_d[kt * P:(kt + 1) * P, :])
                return t

            # SBUF fp32 accumulators, one per (quarter, unit chunk).
            acc = [
                [
                    apool.tile([P, QB], f32, tag=f"acc{q}_{uc}", name=f"acc{q}_{uc}")
                    for uc in range(UC)
                ]
                for q in range(NQ)
            ]

            # Window 0's W tiles up front (they arrive during warmup).
            wtiles = [w_dma(0, j) for j in range(WINS[0])]

            # Last window's PSUM tiles are allocated (and ACC-preloaded)
            # during window NW-2's epilogues, so the preload copies sit
            # right behind that window's ACC adds in the DVE queue and
            # stay off the PE critical path.
            preloaded = {}

            for wi in range(NW):
                L = WINS[wi]
                last_win = wi == NW - 1
                # Pace next window's W DMAs uniformly over this window's
                # 4*L kt-steps.
                nxt = WINS[wi + 1] if not last_win else 0
                nxt_tiles = [None] * nxt
                steps = 4 * L
                issued = 0
                step = 0
                for q in range(NQ):
                    if last_win:
                        psums = preloaded[q]
                    else:
                        psums = [
                            mpsum.tile(
                                [P, QB], f32,
                                tag=f"ps{4 * (wi % 2) + uc}",
                                name=f"ps{wi}_{q}_{uc}",
                            )
                            for uc in range(UC)
                        ]
                    for j in range(L):
                        kt = offs[wi] + j
                        want = ((step + 1) * nxt) // steps
                        while issued < want:
                            nxt_tiles[issued] = w_dma(wi + 1, issued)
                            issued += 1
                        step += 1
                        xt = xpool.tile([P, QB], f16, tag="xt", name=f"x{kt}_{q}")
                        nc.sync.dma_start(
                            xt[:], xt_d[kt * P:(kt + 1) * P, q * QB:(q + 1) * QB]
                        )
                        for uc in range(UC):
                            nc.tensor.matmul(
                                psums[uc][:],
                                wtiles[j][:, uc * P:(uc + 1) * P],
                                xt[:],
                                start=(j == 0 and not last_win),
                                stop=(j == L - 1),
                                skip_group_check=last_win,
                            )
                    if last_win:
                        # Fused epilogue: tanh(psum + bias) -> fp16, straight
                        # from PSUM on the Activation engine.
                        for uc in range(UC):
                            ot = opool.tile([P, QB], f16, tag="ot", name=f"o{q}_{uc}")
                            nc.scalar.activation(
                                ot[:], psums[uc][:], Tanh,
                                bias=bias_t[uc][:], scale=1.0,
                            )
                            nc.sync.dma_start(
                                o_d[uc * P:(uc + 1) * P, q * QB:(q + 1) * QB],
                                ot[:],
                            )
                    else:
                        for uc in range(UC):
                            if wi == 0:
                                nc.vector.tensor_copy(acc[q][uc][:], psums[uc][:])
                            else:
                                nc.vector.tensor_tensor(
                                    acc[q][uc][:], acc[q][uc][:], psums[uc][:],
                                    add_op,
                                )
                        if wi == NW - 2:
                            # Allocate + ACC-preload the last window's banks
                            # for this quarter now: the copies run on the
                            # DVE right after this quarter's adds, well
                            # before the last window's matmuls need them.
                            lps = [
                                mpsum.tile(
                                    [P, QB], f32,
                                    tag=f"ps{4 * ((NW - 1) % 2) + uc}",
                                    name=f"ps{NW - 1}_{q}_{uc}",
                                )
                                for uc in range(UC)
                            ]
                            for uc in range(UC):
                                nc.vector.tensor_copy(lps[uc][:], acc[q][uc][:])
                            preloaded[q] = lps
                wtiles = nxt_tiles

    nc.compile()
    return nc


def _get_nc(key=("v3",)):
    if key not in _NC_CACHE:
        _NC_CACHE[key] = _build_nc()
    return _NC_CACHE[key]


def kernel(x, kernel_vector, bias, nonzero_ind):
    global LAST_RESULT
    from concourse.bass_utils import run_bass_kernel_spmd

    x = np.asarray(x, dtype=np.float32)
    kernel_vector = np.asarray(kernel_vector, dtype=np.float32)
    bias = np.asarray(bias, dtype=np.float32)
    nonzero_ind = np.asarray(nonzero_ind)

    nc = _get_nc()

    # Host scatter: dense fp16 weights, K padded to 157*128.
    rows = nonzero_ind[:, 0].astype(np.int64)
    cols = nonzero_ind[:, 1].astype(np.int64)
    w_full = np.zeros(KPAD * U, np.float32)
    np.add.at(w_full, rows * U + cols, kernel_vector)
    w_full = w_full.reshape(KPAD, U).astype(np.float16)

    # Shared transposed x, fp16, K-padded.
    xt = np.zeros((KPAD, B), np.float16)
    xt[:K] = x.astype(np.float16).T

    in_maps = []
    for c in range(8):
        in_maps.append({
            "xt_sh": xt,
            "w_sh": np.ascontiguousarray(w_full[:, c * U_SH:(c + 1) * U_SH]),
            "b_sh": np.ascontiguousarray(
                bias[c * U_SH:(c + 1) * U_SH].reshape(UC, P, 1)
            ),
        })

    kwargs = {}
    if TRACE:
        kwargs = dict(trace=True, trace_cores=list(range(8)))
    res = run_bass_kernel_spmd(nc, in_maps, core_ids=list(range(8)), **kwargs)
    LAST_RESULT = res

    # Device already applied bias + tanh; just assemble (out is [U, B]).
    out_t = np.concatenate(
        [res.results[c]["out_p"] for c in range(8)], axis=0
    )
    return out_t.T.astype(np.float32)


# revision 10
# speedup vs baseline: 1.0416x; 1.0091x over previous
"""Trainium2 kernel for: out = tanh(x @ scatter_nd(nonzero_ind, kernel_vector, (20000, 4096)) + bias).

Strategy (8 NeuronCores):
  - Host builds the dense (20000, 4096) fp16 weight matrix from the COO
    triples, pads K to 157*128 = 20096, and pre-transposes x to fp16
    xT (20096, 2048), shared by all cores.
  - Shard units 8-ways: core c owns output columns [c*512, (c+1)*512).
    No K-split => 157 K-tiles (vs 160 for a padded K-quartering), and no
    host-side partial summation.
  - Output-stationary windowed accumulation: K is processed in 8 growing
    windows [4,6,10,16,26,40,40,15]. For each window, 4 batch passes of
    512 columns run over the window's K-tiles, accumulating in 4 PSUM
    banks; the bank set ping-pongs on GLOBAL quarter parity so each
    quarter's epilogue has a full quarter of slack. Window partials are
    folded into SBUF fp32 accumulators, split across the DVE and Pool
    engines (2 tiles each) to halve epilogue latency. Growing windows
    let the W panel stream at a flat ~60 GB/s alongside the 150 GB/s x
    stream instead of demanding 300+ GB/s up front (which starves the
    PE: HBM tops out ~230-290 GB/s effective).
  - The last window preloads ACC back into PSUM and matmuls continue
    with start=False, so the final epilogue reads PSUM directly: fused
    out = tanh(psum + bias) on the Activation engine, fp16 output.
  - PE warmup: a few dummy matmuls on a memset tile cover the initial
    DMA fill so the PE p-state ramp (-> 2.4 GHz after ~3us) completes
    before real work starts; they park in the bank that real work
    touches last (second quarter, last unit chunk).
  - Host only concatenates/transposes the 8 shards.
"""

import numpy as np

P = 128
B, K, U = 2048, 20000, 4096
KT = 157                 # ceil(20000 / 128)
KPAD = KT * P            # 20096
U_SH = U // 8            # 512 units per core
UC = U_SH // P           # 4 unit chunks per core
QB = 512                 # batch quarter width
NQ = B // QB             # 4 batch passes
WINS = [4, 6, 10, 16, 26, 40, 40, 15]   # K-window sizes (sum = 157)
N_WARM = 10              # warmup matmuls (free dim 256) covering DMA fill
WARM_F = 256

TRACE = False            # set by test harness for profiled runs
LAST_RESULT = None       # BassKernelResults of the last run (for the harness)

_NC_CACHE = {}


def _build_nc():
    from concourse import bacc
    import concourse.mybir as mybir
    import concourse.tile as tile

    f32 = mybir.dt.float32
    f16 = mybir.dt.float16
    Tanh = mybir.ActivationFunctionType.Tanh
    add_op = mybir.AluOpType.add

    assert sum(WINS) == KT
    NW = len(WINS)
    offs = [sum(WINS[:i]) for i in range(NW)]

    nc = bacc.Bacc("TRN2", target_bir_lowering=False, debug=False)
    xt_d = nc.dram_tensor("xt_sh", [KPAD, B], f16, kind="ExternalInput").ap()
    w_d = nc.dram_tensor("w_sh", [KPAD, U_SH], f16, kind="ExternalInput").ap()
    b_d = nc.dram_tensor("b_sh", [UC, P, 1], f32, kind="ExternalInput").ap()
    o_d = nc.dram_tensor("out_p", [U_SH, B], f16, kind="ExternalOutput").ap()

    with tile.TileContext(nc) as tc:
        with (
            tc.tile_pool(name="wpanel", bufs=3) as wpool,
            tc.tile_pool(name="xstream", bufs=16) as xpool,
            tc.tile_pool(name="accpool", bufs=1) as apool,
            tc.tile_pool(name="bias", bufs=1) as bpool,
            tc.tile_pool(name="ostage", bufs=4) as opool,
            tc.tile_pool(name="fstage", bufs=4) as spool,
            tc.tile_pool(name="warm", bufs=1) as warmpool,
            tc.tile_pool(name="mpsum", bufs=1, space="PSUM") as mpsum,
        ):
            # Bank set for global quarter index g: tags ps{4*(g%2)+uc}.
            def psum_set(g, label):
                return [
                    mpsum.tile(
                        [P, QB], f32,
                        tag=f"ps{4 * (g % 2) + uc}",
                        name=f"ps_{label}_{uc}",
                    )
                    for uc in range(UC)
                ]

            # Epilogue work is split across engines (Pool cannot touch
            # PSUM): even unit chunks fold on the DVE directly; odd ones
            # stage PSUM->SBUF on the Activation engine, then add on Pool.
            def fold(q, uc, psum, first):
                if uc % 2 == 0:
                    if first:
                        nc.vector.tensor_copy(acc[q][uc][:], psum[:])
                    else:
                        nc.vector.tensor_tensor(
                            acc[q][uc][:], acc[q][uc][:], psum[:], add_op
                        )
                else:
                    if first:
                        nc.scalar.copy(acc[q][uc][:], psum[:])
                    else:
                        st = spool.tile([P, QB], f32, tag="st", name=f"st{q}_{uc}")
                        nc.scalar.copy(st[:], psum[:])
                        nc.gpsimd.tensor_tensor(
                            acc[q][uc][:], acc[q][uc][:], st[:], add_op
                        )

            def preload_copy(psum, q, uc):
                if uc % 2 == 0:
                    nc.vector.tensor_copy(psum[:], acc[q][uc][:])
                else:
                    nc.scalar.copy(psum[:], acc[q][uc][:])

            # PE warmup: ramp the tensor-engine p-state while the first
            # real tiles stream in. Parks in bank ps7 (used by quarter
            # g=1, uc=3 — the last bank real work touches).
            warm = warmpool.tile([P, WARM_F], f16, tag="warm", name="warm")
            nc.vector.memset(warm[:], 0.0)
            warm_ps = mpsum.tile([P, WARM_F], f32, tag="ps7", name="warm_ps")
            for _ in range(N_WARM):
                nc.tensor.matmul(
                    warm_ps[:], warm[:, :P], warm[:], start=True, stop=True
                )

            # Per-partition bias chunks (psum partition dim = units).
            bias_t = []
            for uc in range(UC):
                bt = bpool.tile([P, 1], f32, tag=f"b{uc}", name=f"b{uc}")
                nc.sync.dma_start(bt[:], b_d[uc])
                bias_t.append(bt)

            # W window-slot tiles: tag per slot, 3 deep so the next
            # window's prefetch never waits on the current window.
            def w_dma(wi, j):
                kt = offs[wi] + j
                t = wpool.tile([P, U_SH], f16, tag=f"w{j}", name=f"w{wi}_{j}")
                nc.sync.dma_start(t[:], w_d[kt * P:(kt + 1) * P, :])
                return t

            # SBUF fp32 accumulators, one per (quarter, unit chunk).
            acc = [
                [
                    apool.tile([P, QB], f32, tag=f"acc{q}_{uc}", name=f"acc{q}_{uc}")
                    for uc in range(UC)
                ]
                for q in range(NQ)
            ]

            # Window 0's W tiles up front (they arrive during warmup).
            wtiles = [w_dma(0, j) for j in range(WINS[0])]

            # Last window's PSUM tiles for quarters 0/1 are allocated and
            # ACC-preloaded during window NW-2's same-parity epilogues
            # (quarters 2/3), so the copies ride right behind those adds.
            # Quarters 2/3 of the last window preload at quarter top (they
            # wait on the final-activation reads of quarters 0/1, which
            # happen a full quarter earlier).
            preloaded = {}

            for wi in range(NW):
                L = WINS[wi]
                last_win = wi == NW - 1
                nxt = WINS[wi + 1] if not last_win else 0
                nxt_tiles = [None] * nxt
                steps = 4 * L
                issued = 0
                step = 0
                for q in range(NQ):
                    g = 4 * wi + q
                    if last_win:
                        if q in preloaded:
                            psums = preloaded.pop(q)
                        else:
                            psums = psum_set(g, f"{wi}_{q}")
                            for uc in range(UC):
                                preload_copy(psums[uc], q, uc)
                    else:
                        psums = psum_set(g, f"{wi}_{q}")
                    for j in range(L):
                        kt = offs[wi] + j
                        want = ((step + 1) * nxt) // steps
                        while issued < want:
                            nxt_tiles[issued] = w_dma(wi + 1, issued)
                            issued += 1
                        step += 1
                        xt = xpool.tile([P, QB], f16, tag="xt", name=f"x{kt}_{q}")
                        nc.sync.dma_start(
                            xt[:], xt_d[kt * P:(kt + 1) * P, q * QB:(q + 1) * QB]
                        )
                        for uc in range(UC):
                            nc.tensor.matmul(
                                psums[uc][:],
                                wtiles[j][:, uc * P:(uc + 1) * P],
                                xt[:],
                                start=(j == 0 and not last_win),
                                stop=(j == L - 1),
                                skip_group_check=last_win,
                            )
                    if last_win:
                        # Fused epilogue: tanh(psum + bias) -> fp16, straight
                        # from PSUM on the Activation engine.
                        for uc in range(UC):
                            ot = opool.tile([P, QB], f16, tag="ot", name=f"o{q}_{uc}")
                            nc.scalar.activation(
                                ot[:], psums[uc][:], Tanh,
                                bias=bias_t[uc][:], scale=1.0,
                            )
                            nc.sync.dma_start(
                                o_d[uc * P:(uc + 1) * P, q * QB:(q + 1) * QB],
                                ot[:],
                            )
                    else:
                        for uc in range(UC):
                            fold(q, uc, psums[uc], wi == 0)
                        if wi == NW - 2 and q >= 2:
                            # Preload the last window's same-parity quarter
                            # (q-2) right behind this quarter's adds.
                            lq = q - 2
                            lps = psum_set(4 * (NW - 1) + lq, f"{NW - 1}_{lq}")
                            for uc in range(UC):
                                preload_copy(lps[uc], lq, uc)
                            preloaded[lq] = lps
                wtiles = nxt_tiles

    nc.compile()
    return nc


def _get_nc(key=("v4",)):
    if key not in _NC_CACHE:
        _NC_CACHE[key] = _build_nc()
    return _NC_CACHE[key]


def kernel(x, kernel_vector, bias, nonzero_ind):
    global LAST_RESULT
    from concourse.bass_utils import run_bass_kernel_spmd

    x = np.asarray(x, dtype=np.float32)
    kernel_vector = np.asarray(kernel_vector, dtype=np.float32)
    bias = np.asarray(bias, dtype=np.float32)
    nonzero_ind = np.asarray(nonzero_ind)

    nc = _get_nc()

    # Host scatter: dense fp16 weights, K padded to 157*128.
    rows = nonzero_ind[:, 0].astype(np.int64)
    cols = nonzero_ind[:, 1].astype(np.int64)
    w_full = np.zeros(KPAD * U, np.float32)
    np.add.at(w_full, rows * U + cols, kernel_vector)
    w_full = w_full.reshape(KPAD, U).astype(np.float16)

    # Shared transposed x, fp16, K-padded.
    xt = np.zeros((KPAD, B), np.float16)
    xt[:K] = x.astype(np.float16).T

    in_maps = []
    for c in range(8):
        in_maps.append({
            "xt_sh": xt,
            "w_sh": np.ascontiguousarray(w_full[:, c * U_SH:(c + 1) * U_SH]),
            "b_sh": np.ascontiguousarray(
                bias[c * U_SH:(c + 1) * U_SH].reshape(UC, P, 1)
            ),
        })

    kwargs = {}
    if TRACE:
        kwargs = dict(trace=True, trace_cores=list(range(8)))
    res = run_bass_kernel_spmd(nc, in_maps, core_ids=list(range(8)), **kwargs)
    LAST_RESULT = res

    # Device already applied bias + tanh; just assemble (out is [U, B]).
    out_t = np.concatenate(
        [res.results[c]["out_p"] for c in range(8)], axis=0
    )
    return out_t.T.astype(np.float32)


# revision 14
# speedup vs baseline: 1.0819x; 1.0386x over previous
"""Trainium2 kernel for: out = tanh(x @ scatter_nd(nonzero_ind, kernel_vector, (20000, 4096)) + bias).

Strategy (8 NeuronCores):
  - Host builds the dense (20000, 4096) fp16 weight matrix from the COO
    triples, pads K to 157*128 = 20096, and pre-transposes x to fp16
    xT (20096, 2048), shared by all cores.
  - Shard units 8-ways: core c owns output columns [c*512, (c+1)*512).
    No K-split => 157 K-tiles (vs 160 for a padded K-quartering), and no
    host-side partial summation.
  - Output-stationary windowed accumulation: K is processed in 8 growing
    windows [4,6,10,16,26,40,40,15] so the W panel streams in at a flat
    ~55 GB/s alongside the ~150 GB/s x stream instead of demanding
    300+ GB/s up front. Window partials accumulate in PSUM and are
    folded into SBUF fp32 accumulators between windows.
  - DMA-issue rate is the scarce resource (~0.6us of descriptor-gen per
    dma_start per sequencer): windows 0..5 process batch in 1024-wide
    half-passes (one x DMA feeds 8 matmuls, all 8 PSUM banks in
    rotation), x DMAs own the SP sequencer exclusively, W prefetch
    alternates between the Activation and Pool sequencers, and
    bias/output DMAs ride on the DVE sequencer.
  - Windows 6..7 switch to 512-wide quarter passes (4 banks, parity
    ping-pong): W traffic is negligible by then, transitions are clean,
    and the final epilogue tail is short.
  - Fold engines: even unit chunks on the DVE (direct PSUM access);
    odd ones stage PSUM->SBUF on the Activation engine then add on Pool
    (Pool cannot touch PSUM).
  - The last window preloads ACC back into PSUM and matmuls continue
    with start=False, so the final epilogue reads PSUM directly: fused
    out = tanh(psum + bias) on the Activation engine, fp16 output.
  - PE warmup: a few dummy matmuls on a memset tile keep the PE busy
    through the initial DMA fill so its p-state ramp (-> 2.4 GHz)
    completes before real work starts.
  - Host only concatenates/transposes the 8 shards.
"""

import numpy as np

P = 128
B, K, U = 2048, 20000, 4096
KT = 157                 # ceil(20000 / 128)
KPAD = KT * P            # 20096
U_SH = U // 8            # 512 units per core
UC = U_SH // P           # 4 unit chunks per core
QB = 512                 # batch block width (PSUM bank free size)
NB = B // QB             # 4 batch blocks
WINS = [4, 6, 10, 16, 26, 40, 40, 15]   # K-window sizes (sum = 157)
N_HALF_WINS = 6          # windows [0..6) use 1024-wide half passes
N_WARM = 8               # warmup matmuls (free dim 256) covering DMA fill
WARM_F = 256

TRACE = False            # set by test harness for profiled runs
LAST_RESULT = None       # BassKernelResults of the last run (for the harness)

_NC_CACHE = {}


def _build_nc():
    from concourse import bacc
    import concourse.mybir as mybir
    import concourse.tile as tile

    f32 = mybir.dt.float32
    f16 = mybir.dt.float16
    Tanh = mybir.ActivationFunctionType.Tanh
    add_op = mybir.AluOpType.add

    assert sum(WINS) == KT
    NW = len(WINS)
    offs = [sum(WINS[:i]) for i in range(NW)]

    nc = bacc.Bacc("TRN2", target_bir_lowering=False, debug=False)
    xt_d = nc.dram_tensor("xt_sh", [KPAD, B], f16, kind="ExternalInput").ap()
    w_d = nc.dram_tensor("w_sh", [KPAD, U_SH], f16, kind="ExternalInput").ap()
    b_d = nc.dram_tensor("b_sh", [UC, P, 1], f32, kind="ExternalInput").ap()
    o_d = nc.dram_tensor("out_p", [U_SH, B], f16, kind="ExternalOutput").ap()

    with tile.TileContext(nc) as tc:
        with (
            tc.tile_pool(name="wpanel", bufs=2) as wpool,
            tc.tile_pool(name="xhalf", bufs=10) as xhpool,
            tc.tile_pool(name="xquart", bufs=12) as xqpool,
            tc.tile_pool(name="accpool", bufs=1) as apool,
            tc.tile_pool(name="bias", bufs=1) as bpool,
            tc.tile_pool(name="ostage", bufs=4) as opool,
            tc.tile_pool(name="fstage", bufs=4) as spool,
            tc.tile_pool(name="warm", bufs=1) as warmpool,
            tc.tile_pool(name="mpsum", bufs=1, space="PSUM") as mpsum,
        ):
            # ---- epilogue helpers (engine-split; Pool can't touch PSUM) --
            def fold(b, uc, psum, first):
                if uc % 2 == 0:
                    if first:
                        nc.vector.tensor_copy(acc[b][uc][:], psum[:])
                    else:
                        nc.vector.tensor_tensor(
                            acc[b][uc][:], acc[b][uc][:], psum[:], add_op
                        )
                else:
                    if first:
                        nc.scalar.copy(acc[b][uc][:], psum[:])
                    else:
                        st = spool.tile([P, QB], f32, tag="st", name=f"st{b}_{uc}")
                        nc.scalar.copy(st[:], psum[:])
                        nc.gpsimd.tensor_tensor(
                            acc[b][uc][:], acc[b][uc][:], st[:], add_op
                        )

            def preload_copy(psum, b, uc):
                if uc % 2 == 0:
                    nc.vector.tensor_copy(psum[:], acc[b][uc][:])
                else:
                    nc.scalar.copy(psum[:], acc[b][uc][:])

            # PE warmup (parks in bank ps7, real work reaches it last).
            warm = warmpool.tile([P, WARM_F], f16, tag="warm", name="warm")
            nc.vector.memset(warm[:], 0.0)
            warm_ps = mpsum.tile([P, WARM_F], f32, tag="ps7", name="warm_ps")
            for _ in range(N_WARM):
                nc.tensor.matmul(
                    warm_ps[:], warm[:, :P], warm[:], start=True, stop=True
                )

            # W window-slot tiles; issue alternates Activation/Pool
            # sequencers to keep SP free for the x stream.
            def w_dma(wi, j):
                kt = offs[wi] + j
                t = wpool.tile([P, U_SH], f16, tag=f"w{j}", name=f"w{wi}_{j}")
                eng = nc.scalar if j % 2 == 0 else nc.gpsimd
                eng.dma_start(t[:], w_d[kt * P:(kt + 1) * P, :])
                return t

            # SBUF fp32 accumulators, one per (batch block, unit chunk).
            acc = [
                [
                    apool.tile([P, QB], f32, tag=f"acc{b}_{uc}", name=f"acc{b}_{uc}")
                    for uc in range(UC)
                ]
                for b in range(NB)
            ]

            def psum_quarter(q, label):
                # Quarter pass: 4 banks, parity ping-pong.
                return [
                    mpsum.tile(
                        [P, QB], f32,
                        tag=f"ps{4 * (q % 2) + uc}",
                        name=f"ps_{label}_{uc}",
                    )
                    for uc in range(UC)
                ]

            # Window 0's W tiles up front (they arrive during warmup).
            wtiles = [w_dma(0, j) for j in range(WINS[0])]

            # Per-partition bias chunks (psum partition dim = units);
            # issued after window 0's W so they don't delay it.
            bias_t = []
            for uc in range(UC):
                bt = bpool.tile([P, 1], f32, tag=f"b{uc}", name=f"b{uc}")
                nc.scalar.dma_start(bt[:], b_d[uc])
                bias_t.append(bt)
            preloaded = {}

            for wi in range(NW):
                L = WINS[wi]
                last_win = wi == NW - 1
                halves = wi < N_HALF_WINS
                nxt = WINS[wi + 1] if not last_win else 0
                steps = (2 if halves else 4) * L
                nxt_tiles = [None] * nxt
                issued = 0
                step = 0

                def pace_w(wi=wi):
                    nonlocal issued, step
                    want = ((step + 1) * nxt) // steps
                    while issued < want:
                        nxt_tiles[issued] = w_dma(wi + 1, issued)
                        issued += 1
                    step += 1

                if halves:
                    for h in range(2):
                        # All 8 banks: (uc, sub) -> ps{2*uc+sub}.
                        psums = [
                            [
                                mpsum.tile(
                                    [P, QB], f32,
                                    tag=f"ps{2 * uc + s}",
                                    name=f"ps_{wi}_{h}_{uc}_{s}",
                                )
                                for s in range(2)
                            ]
                            for uc in range(UC)
                        ]
                        for j in range(L):
                            kt = offs[wi] + j
                            pace_w()
                            xt = xhpool.tile(
                                [P, 2 * QB], f16, tag="xh", name=f"xh{kt}_{h}"
                            )
                            nc.sync.dma_start(
                                xt[:],
                                xt_d[kt * P:(kt + 1) * P,
                                     h * 2 * QB:(h + 1) * 2 * QB],
                            )
                            for uc in range(UC):
                                for s in range(2):
                                    nc.tensor.matmul(
                                        psums[uc][s][:],
                                        wtiles[j][:, uc * P:(uc + 1) * P],
                                        xt[:, s * QB:(s + 1) * QB],
                                        start=(j == 0),
                                        stop=(j == L - 1),
                                    )
                        for uc in range(UC):
                            for s in range(2):
                                fold(2 * h + s, uc, psums[uc][s], wi == 0)
                else:
                    for q in range(NB):
                        if last_win:
                            if q in preloaded:
                                psums = preloaded.pop(q)
                            else:
                                psums = psum_quarter(q, f"{wi}_{q}")
                                for uc in range(UC):
                                    preload_copy(psums[uc], q, uc)
                        else:
                            psums = psum_quarter(q, f"{wi}_{q}")
                        for j in range(L):
                            kt = offs[wi] + j
                            pace_w()
                            xt = xqpool.tile(
                                [P, QB], f16, tag="xq", name=f"xq{kt}_{q}"
                            )
                            nc.sync.dma_start(
                                xt[:],
                                xt_d[kt * P:(kt + 1) * P, q * QB:(q + 1) * QB],
                            )
                            for uc in range(UC):
                                nc.tensor.matmul(
                                    psums[uc][:],
                                    wtiles[j][:, uc * P:(uc + 1) * P],
                                    xt[:],
                                    start=(j == 0 and not last_win),
                                    stop=(j == L - 1),
                                    skip_group_check=last_win,
                                )
                        if last_win:
                            # Fused epilogue: tanh(psum + bias) -> fp16,
                            # straight from PSUM on the Activation engine.
                            for uc in range(UC):
                                ot = opool.tile(
                                    [P, QB], f16, tag="ot", name=f"o{q}_{uc}"
                                )
                                nc.scalar.activation(
                                    ot[:], psums[uc][:], Tanh,
                                    bias=bias_t[uc][:], scale=1.0,
                                )
                                nc.scalar.dma_start(
                                    o_d[uc * P:(uc + 1) * P,
                                        q * QB:(q + 1) * QB],
                                    ot[:],
                                )
                        else:
                            for uc in range(UC):
                                fold(q, uc, psums[uc], False)
                            if wi == NW - 2 and q >= 2:
                                # Preload the last window's same-parity
                                # quarter right behind these folds.
                                lq = q - 2
                                lps = psum_quarter(lq, f"{NW - 1}_{lq}")
                                for uc in range(UC):
                                    preload_copy(lps[uc], lq, uc)
                                preloaded[lq] = lps
                wtiles = nxt_tiles

    nc.compile()
    return nc


def _get_nc(key=("v5",)):
    if key not in _NC_CACHE:
        _NC_CACHE[key] = _build_nc()
    return _NC_CACHE[key]


def kernel(x, kernel_vector, bias, nonzero_ind):
    global LAST_RESULT
    from concourse.bass_utils import run_bass_kernel_spmd

    x = np.asarray(x, dtype=np.float32)
    kernel_vector = np.asarray(kernel_vector, dtype=np.float32)
    bias = np.asarray(bias, dtype=np.float32)
    nonzero_ind = np.asarray(nonzero_ind)

    nc = _get_nc()

    # Host scatter: dense fp16 weights, K padded to 157*128.
    rows = nonzero_ind[:, 0].astype(np.int64)
    cols = nonzero_ind[:, 1].astype(np.int64)
    w_full = np.zeros(KPAD * U, np.float32)
    np.add.at(w_full, rows * U + cols, kernel_vector)
    w_full = w_full.reshape(KPAD, U).astype(np.float16)

    # Shared transposed x, fp16, K-padded.
    xt = np.zeros((KPAD, B), np.float16)
    xt[:K] = x.astype(np.float16).T

    in_maps = []
    for c in range(8):
        in_maps.append({
            "xt_sh": xt,
            "w_sh": np.ascontiguousarray(w_full[:, c * U_SH:(c + 1) * U_SH]),
            "b_sh": np.ascontiguousarray(
                bias[c * U_SH:(c + 1) * U_SH].reshape(UC, P, 1)
            ),
        })

    kwargs = {}
    if TRACE:
        kwargs = dict(trace=True, trace_cores=list(range(8)))
    res = run_bass_kernel_spmd(nc, in_maps, core_ids=list(range(8)), **kwargs)
    LAST_RESULT = res

    # Device already applied bias + tanh; just assemble (out is [U, B]).
    out_t = np.concatenate(
        [res.results[c]["out_p"] for c in range(8)], axis=0
    )
    return out_t.T.astype(np.float32)


# revision 15
# speedup vs baseline: 1.0870x; 1.0048x over previous
"""Trainium2 kernel for: out = tanh(x @ scatter_nd(nonzero_ind, kernel_vector, (20000, 4096)) + bias).

Strategy (8 NeuronCores):
  - Host builds the dense (20000, 4096) fp16 weight matrix from the COO
    triples, pads K to 157*128 = 20096, and pre-transposes x to fp16
    xT (20096, 2048), shared by all cores.
  - Shard units 8-ways: core c owns output columns [c*512, (c+1)*512).
    No K-split => 157 K-tiles (vs 160 for a padded K-quartering), and no
    host-side partial summation.
  - Output-stationary windowed accumulation: K is processed in 8 growing
    windows [4,6,10,16,26,40,40,15] so the W panel streams in at a flat
    ~55 GB/s alongside the ~150 GB/s x stream instead of demanding
    300+ GB/s up front. Window partials accumulate in PSUM and are
    folded into SBUF fp32 accumulators between windows.
  - DMA-issue rate is the scarce resource (~0.6us of descriptor-gen per
    dma_start per sequencer): windows 0..5 process batch in 1024-wide
    half-passes (one x DMA feeds 8 matmuls, all 8 PSUM banks in
    rotation), x DMAs own the SP sequencer exclusively, W prefetch
    alternates between the Activation and Pool sequencers, and
    bias/output DMAs ride on the DVE sequencer.
  - Windows 6..7 switch to 512-wide quarter passes (4 banks, parity
    ping-pong): W traffic is negligible by then, transitions are clean,
    and the final epilogue tail is short.
  - Fold engines: even unit chunks on the DVE (direct PSUM access);
    odd ones stage PSUM->SBUF on the Activation engine then add on Pool
    (Pool cannot touch PSUM).
  - The last window preloads ACC back into PSUM and matmuls continue
    with start=False, so the final epilogue reads PSUM directly: fused
    out = tanh(psum + bias) on the Activation engine, fp16 output.
  - PE warmup: a few dummy matmuls on a memset tile keep the PE busy
    through the initial DMA fill so its p-state ramp (-> 2.4 GHz)
    completes before real work starts.
  - Host only concatenates/transposes the 8 shards.
"""

import numpy as np

P = 128
B, K, U = 2048, 20000, 4096
KT = 157                 # ceil(20000 / 128)
KPAD = KT * P            # 20096
U_SH = U // 8            # 512 units per core
UC = U_SH // P           # 4 unit chunks per core
QB = 512                 # batch block width (PSUM bank free size)
NB = B // QB             # 4 batch blocks
WINS = [4, 6, 10, 16, 26, 40, 40, 15]   # K-window sizes (sum = 157)
N_HALF_WINS = 6          # windows [0..6) use 1024-wide half passes
N_WARM = 8               # warmup matmuls (free dim 256) covering DMA fill
WARM_F = 256

TRACE = False            # set by test harness for profiled runs
LAST_RESULT = None       # BassKernelResults of the last run (for the harness)

_NC_CACHE = {}


def _build_nc():
    from concourse import bacc
    import concourse.mybir as mybir
    import concourse.tile as tile

    f32 = mybir.dt.float32
    f16 = mybir.dt.float16
    Tanh = mybir.ActivationFunctionType.Tanh
    add_op = mybir.AluOpType.add

    assert sum(WINS) == KT
    NW = len(WINS)
    offs = [sum(WINS[:i]) for i in range(NW)]

    nc = bacc.Bacc("TRN2", target_bir_lowering=False, debug=False)
    xt_d = nc.dram_tensor("xt_sh", [KPAD, B], f16, kind="ExternalInput").ap()
    w_d = nc.dram_tensor("w_sh", [KPAD, U_SH], f16, kind="ExternalInput").ap()
    b_d = nc.dram_tensor("b_sh", [UC, P, 1], f32, kind="ExternalInput").ap()
    o_d = nc.dram_tensor("out_p", [U_SH, B], f16, kind="ExternalOutput").ap()

    with tile.TileContext(nc) as tc:
        with (
            tc.tile_pool(name="wpanel", bufs=2) as wpool,
            tc.tile_pool(name="xhalf", bufs=10) as xhpool,
            tc.tile_pool(name="xquart", bufs=12) as xqpool,
            tc.tile_pool(name="accpool", bufs=1) as apool,
            tc.tile_pool(name="bias", bufs=1) as bpool,
            tc.tile_pool(name="ostage", bufs=4) as opool,
            tc.tile_pool(name="fstage", bufs=4) as spool,
            tc.tile_pool(name="warm", bufs=1) as warmpool,
            tc.tile_pool(name="mpsum", bufs=1, space="PSUM") as mpsum,
        ):
            # ---- epilogue helpers (engine-split; Pool can't touch PSUM) --
            def fold(b, uc, psum, first):
                if uc % 2 == 0:
                    if first:
                        nc.vector.tensor_copy(acc[b][uc][:], psum[:])
                    else:
                        nc.vector.tensor_tensor(
                            acc[b][uc][:], acc[b][uc][:], psum[:], add_op
                        )
                else:
                    if first:
                        nc.scalar.copy(acc[b][uc][:], psum[:])
                    else:
                        st = spool.tile([P, QB], f32, tag="st", name=f"st{b}_{uc}")
                        nc.scalar.copy(st[:], psum[:])
                        nc.gpsimd.tensor_tensor(
                            acc[b][uc][:], acc[b][uc][:], st[:], add_op
                        )

            def preload_copy(psum, b, uc):
                if uc % 2 == 0:
                    nc.vector.tensor_copy(psum[:], acc[b][uc][:])
                else:
                    nc.scalar.copy(psum[:], acc[b][uc][:])

            # PE warmup (parks in bank ps7, real work reaches it last).
            warm = warmpool.tile([P, WARM_F], f16, tag="warm", name="warm")
            nc.vector.memset(warm[:], 0.0)
            warm_ps = mpsum.tile([P, WARM_F], f32, tag="ps7", name="warm_ps")
            for _ in range(N_WARM):
                nc.tensor.matmul(
                    warm_ps[:], warm[:, :P], warm[:], start=True, stop=True
                )

            # W window-slot tiles; issued from the Activation sequencer to
            # keep SP free for the x stream. (Pool-issued DMAs go through
            # SWDGE, which doesn't start executing until tens of us in.)
            def w_dma(wi, j):
                kt = offs[wi] + j
                t = wpool.tile([P, U_SH], f16, tag=f"w{j}", name=f"w{wi}_{j}")
                nc.scalar.dma_start(t[:], w_d[kt * P:(kt + 1) * P, :])
                return t

            # SBUF fp32 accumulators, one per (batch block, unit chunk).
            acc = [
                [
                    apool.tile([P, QB], f32, tag=f"acc{b}_{uc}", name=f"acc{b}_{uc}")
                    for uc in range(UC)
                ]
                for b in range(NB)
            ]

            def psum_quarter(q, label):
                # Quarter pass: 4 banks, parity ping-pong.
                return [
                    mpsum.tile(
                        [P, QB], f32,
                        tag=f"ps{4 * (q % 2) + uc}",
                        name=f"ps_{label}_{uc}",
                    )
                    for uc in range(UC)
                ]

            # Window 0's W tiles up front (they arrive during warmup).
            wtiles = [w_dma(0, j) for j in range(WINS[0])]

            # Per-partition bias chunks (psum partition dim = units);
            # issued after window 0's W so they don't delay it.
            bias_t = []
            for uc in range(UC):
                bt = bpool.tile([P, 1], f32, tag=f"b{uc}", name=f"b{uc}")
                nc.scalar.dma_start(bt[:], b_d[uc])
                bias_t.append(bt)
            preloaded = {}

            for wi in range(NW):
                L = WINS[wi]
                last_win = wi == NW - 1
                halves = wi < N_HALF_WINS
                nxt = WINS[wi + 1] if not last_win else 0
                steps = (2 if halves else 4) * L
                nxt_tiles = [None] * nxt
                issued = 0
                step = 0

                def pace_w(wi=wi):
                    nonlocal issued, step
                    want = ((step + 1) * nxt) // steps
                    while issued < want:
                        nxt_tiles[issued] = w_dma(wi + 1, issued)
                        issued += 1
                    step += 1

                if halves:
                    for h in range(2):
                        # All 8 banks: (uc, sub) -> ps{2*uc+sub}.
                        psums = [
                            [
                                mpsum.tile(
                                    [P, QB], f32,
                                    tag=f"ps{2 * uc + s}",
                                    name=f"ps_{wi}_{h}_{uc}_{s}",
                                )
                                for s in range(2)
                            ]
                            for uc in range(UC)
                        ]
                        for j in range(L):
                            kt = offs[wi] + j
                            pace_w()
                            xt = xhpool.tile(
                                [P, 2 * QB], f16, tag="xh", name=f"xh{kt}_{h}"
                            )
                            nc.sync.dma_start(
                                xt[:],
                                xt_d[kt * P:(kt + 1) * P,
                                     h * 2 * QB:(h + 1) * 2 * QB],
                            )
                            for uc in range(UC):
                                for s in range(2):
                                    nc.tensor.matmul(
                                        psums[uc][s][:],
                                        wtiles[j][:, uc * P:(uc + 1) * P],
                                        xt[:, s * QB:(s + 1) * QB],
                                        start=(j == 0),
                                        stop=(j == L - 1),
                                    )
                        for uc in range(UC):
                            for s in range(2):
                                fold(2 * h + s, uc, psums[uc][s], wi == 0)
                else:
                    for q in range(NB):
                        if last_win:
                            if q in preloaded:
                                psums = preloaded.pop(q)
                            else:
                                psums = psum_quarter(q, f"{wi}_{q}")
                                for uc in range(UC):
                                    preload_copy(psums[uc], q, uc)
                        else:
                            psums = psum_quarter(q, f"{wi}_{q}")
                        for j in range(L):
                            kt = offs[wi] + j
                            pace_w()
                            xt = xqpool.tile(
                                [P, QB], f16, tag="xq", name=f"xq{kt}_{q}"
                            )
                            nc.sync.dma_start(
                                xt[:],
                                xt_d[kt * P:(kt + 1) * P, q * QB:(q + 1) * QB],
                            )
                            for uc in range(UC):
                                nc.tensor.matmul(
                                    psums[uc][:],
                                    wtiles[j][:, uc * P:(uc + 1) * P],
                                    xt[:],
                                    start=(j == 0 and not last_win),
                                    stop=(j == L - 1),
                                    skip_group_check=last_win,
                                )
                        if last_win:
                            # Fused epilogue: tanh(psum + bias) -> fp16,
                            # straight from PSUM on the Activation engine.
                            for uc in range(UC):
                                ot = opool.tile(
                                    [P, QB], f16, tag="ot", name=f"o{q}_{uc}"
                                )
                                nc.scalar.activation(
                                    ot[:], psums[uc][:], Tanh,
                                    bias=bias_t[uc][:], scale=1.0,
                                )
                                nc.scalar.dma_start(
                                    o_d[uc * P:(uc + 1) * P,
                                        q * QB:(q + 1) * QB],
                                    ot[:],
                                )
                        else:
                            for uc in range(UC):
                                fold(q, uc, psums[uc], False)
                            if wi == NW - 2 and q >= 2:
                                # Preload the last window's same-parity
                                # quarter right behind these folds.
                                lq = q - 2
                                lps = psum_quarter(lq, f"{NW - 1}_{lq}")
                                for uc in range(UC):
                                    preload_copy(lps[uc], lq, uc)
                                preloaded[lq] = lps
                wtiles = nxt_tiles

    nc.compile()
    return nc


def _get_nc(key=("v5",)):
    if key not in _NC_CACHE:
        _NC_CACHE[key] = _build_nc()
    return _NC_CACHE[key]


def kernel(x, kernel_vector, bias, nonzero_ind):
    global LAST_RESULT
    from concourse.bass_utils import run_bass_kernel_spmd

    x = np.asarray(x, dtype=np.float32)
    kernel_vector = np.asarray(kernel_vector, dtype=np.float32)
    bias = np.asarray(bias, dtype=np.float32)
    nonzero_ind = np.asarray(nonzero_ind)

    nc = _get_nc()

    # Host scatter: dense fp16 weights, K padded to 157*128.
    rows = nonzero_ind[:, 0].astype(np.int64)
    cols = nonzero_ind[:, 1].astype(np.int64)
    w_full = np.zeros(KPAD * U, np.float32)
    np.add.at(w_full, rows * U + cols, kernel_vector)
    w_full = w_full.reshape(KPAD, U).astype(np.float16)

    # Shared transposed x, fp16, K-padded.
    xt = np.zeros((KPAD, B), np.float16)
    xt[:K] = x.astype(np.float16).T

    in_maps = []
    for c in range(8):
        in_maps.append({
            "xt_sh": xt,
            "w_sh": np.ascontiguousarray(w_full[:, c * U_SH:(c + 1) * U_SH]),
            "b_sh": np.ascontiguousarray(
                bias[c * U_SH:(c + 1) * U_SH].reshape(UC, P, 1)
            ),
        })

    kwargs = {}
    if TRACE:
        kwargs = dict(trace=True, trace_cores=list(range(8)))
    res = run_bass_kernel_spmd(nc, in_maps, core_ids=list(range(8)), **kwargs)
    LAST_RESULT = res

    # Device already applied bias + tanh; just assemble (out is [U, B]).
    out_t = np.concatenate(
        [res.results[c]["out_p"] for c in range(8)], axis=0
    )
    return out_t.T.astype(np.float32)


# revision 16
# speedup vs baseline: 1.1158x; 1.0265x over previous
"""Trainium2 kernel for: out = tanh(x @ scatter_nd(nonzero_ind, kernel_vector, (20000, 4096)) + bias).

Strategy (8 NeuronCores):
  - Host builds the dense (20000, 4096) fp16 weight matrix from the COO
    triples, pads K to 157*128 = 20096, pre-transposes x to fp16
    xT (20096, 2048) (shared by all cores), and pre-swizzles each
    core's W shard to partition-major [128, 157, 512] so a whole
    K-window loads as ONE DMA of 128 contiguous 20KB lines.
  - Shard units 8-ways: core c owns output columns [c*512, (c+1)*512).
    No K-split => 157 K-tiles (vs 160 for a padded K-quartering), no
    host-side partial summation.
  - Output-stationary windowed accumulation: K in 8 equal windows
    (7x20 + 17 tiles). Each window runs 4 batch quarter-passes of 512
    columns; a pass accumulates in 4 PSUM banks, ping-ponged by pass
    parity so each pass epilogue has a full pass of slack. Window
    partials fold into SBUF fp32 accumulators (even unit chunks on the
    DVE straight from PSUM; odd ones stage via the Activation engine
    then add on Pool, which cannot touch PSUM).
  - W window tiles: 4 resident in rotation (80 KB/partition); all DMA
    issue is one descriptor-gen per window, so the SP sequencer carries
    only the x stream (sequencer descriptor-gen rate, ~0.6us per
    dma_start, is the scarce resource — per-tile W DMAs starve the PE).
    Window 0 is split into 4 sub-DMAs so the first matmul starts ~9us in.
  - The last window preloads ACC back into PSUM (riding right behind
    window 6's folds) and matmuls continue with start=False, so the
    final epilogue reads PSUM directly: fused out = tanh(psum + bias)
    on the Activation engine, fp16 output.
  - PE warmup: dummy matmuls on a memset tile keep the PE busy through
    the initial DMA fill so its p-state ramp (-> 2.4 GHz) completes
    before real work starts.
  - Host only concatenates/transposes the 8 shards.
"""

import numpy as np

P = 128
B, K, U = 2048, 20000, 4096
KT = 157                 # ceil(20000 / 128)
KPAD = KT * P            # 20096
U_SH = U // 8            # 512 units per core
UC = U_SH // P           # 4 unit chunks per core
QB = 512                 # batch block width (PSUM bank free size)
NB = B // QB             # 4 batch quarters
WINS = [20, 20, 20, 20, 20, 20, 20, 17]   # K-window sizes (sum = 157)
W_RES = 4                # W window tiles resident
N_WARM = 8               # warmup matmuls (free dim 256) covering DMA fill
WARM_F = 256

TRACE = False            # set by test harness for profiled runs
LAST_RESULT = None       # BassKernelResults of the last run (for the harness)

_NC_CACHE = {}


def _build_nc():
    from concourse import bacc
    import concourse.mybir as mybir
    import concourse.tile as tile

    f32 = mybir.dt.float32
    f16 = mybir.dt.float16
    Tanh = mybir.ActivationFunctionType.Tanh
    add_op = mybir.AluOpType.add

    assert sum(WINS) == KT
    NW = len(WINS)
    offs = [sum(WINS[:i]) for i in range(NW)]

    nc = bacc.Bacc("TRN2", target_bir_lowering=False, debug=False)
    xt_d = nc.dram_tensor("xt_sh", [KPAD, B], f16, kind="ExternalInput").ap()
    w_d = nc.dram_tensor("w_sh", [P, KT * U_SH], f16, kind="ExternalInput").ap()
    b_d = nc.dram_tensor("b_sh", [UC, P, 1], f32, kind="ExternalInput").ap()
    o_d = nc.dram_tensor("out_p", [U_SH, B], f16, kind="ExternalOutput").ap()

    with tile.TileContext(nc) as tc:
        with (
            tc.tile_pool(name="wpanel", bufs=1) as wpool,
            tc.tile_pool(name="xquart", bufs=16) as xqpool,
            tc.tile_pool(name="accpool", bufs=1) as apool,
            tc.tile_pool(name="bias", bufs=1) as bpool,
            tc.tile_pool(name="ostage", bufs=4) as opool,
            tc.tile_pool(name="fstage", bufs=4) as spool,
            tc.tile_pool(name="warm", bufs=1) as warmpool,
            tc.tile_pool(name="mpsum", bufs=1, space="PSUM") as mpsum,
        ):
            # ---- epilogue helpers (engine-split; Pool can't touch PSUM) --
            def fold(b, uc, psum, first):
                if uc % 2 == 0:
                    if first:
                        nc.vector.tensor_copy(acc[b][uc][:], psum[:])
                    else:
                        nc.vector.tensor_tensor(
                            acc[b][uc][:], acc[b][uc][:], psum[:], add_op
                        )
                else:
                    if first:
                        nc.scalar.copy(acc[b][uc][:], psum[:])
                    else:
                        st = spool.tile([P, QB], f32, tag="st", name=f"st{b}_{uc}")
                        nc.scalar.copy(st[:], psum[:])
                        nc.gpsimd.tensor_tensor(
                            acc[b][uc][:], acc[b][uc][:], st[:], add_op
                        )

            def preload_copy(psum, b, uc):
                if uc % 2 == 0:
                    nc.vector.tensor_copy(psum[:], acc[b][uc][:])
                else:
                    nc.scalar.copy(psum[:], acc[b][uc][:])

            def psum_quarter(q, label):
                return [
                    mpsum.tile(
                        [P, QB], f32,
                        tag=f"ps{4 * (q % 2) + uc}",
                        name=f"ps_{label}_{uc}",
                    )
                    for uc in range(UC)
                ]

            # PE warmup (parks in bank ps7; real work reaches it late).
            warm = warmpool.tile([P, WARM_F], f16, tag="warm", name="warm")
            nc.vector.memset(warm[:], 0.0)
            warm_ps = mpsum.tile([P, WARM_F], f32, tag="ps7", name="warm_ps")
            for _ in range(N_WARM):
                nc.tensor.matmul(
                    warm_ps[:], warm[:, :P], warm[:], start=True, stop=True
                )

            # W window tiles: one DMA per window (128 lines of L*1KB).
            def w_win_dma(wi, split=1):
                L = WINS[wi]
                t = wpool.tile(
                    [P, L * U_SH], f16, tag=f"w{wi % W_RES}", name=f"ww{wi}"
                )
                c0 = offs[wi] * U_SH
                step = (L + split - 1) // split
                for s in range(0, L, step):
                    n = min(step, L - s)
                    nc.scalar.dma_start(
                        t[:, s * U_SH:(s + n) * U_SH],
                        w_d[:, c0 + s * U_SH:c0 + (s + n) * U_SH],
                    )
                return t

            # First W_RES windows up front; window 0 split for fast start.
            wwin = {}
            wwin[0] = w_win_dma(0, split=4)
            for wi in range(1, W_RES):
                wwin[wi] = w_win_dma(wi)

            # Per-partition bias chunks (psum partition dim = units).
            bias_t = []
            for uc in range(UC):
                bt = bpool.tile([P, 1], f32, tag=f"b{uc}", name=f"b{uc}")
                nc.scalar.dma_start(bt[:], b_d[uc])
                bias_t.append(bt)

            # SBUF fp32 accumulators, one per (batch quarter, unit chunk).
            acc = [
                [
                    apool.tile([P, QB], f32, tag=f"acc{b}_{uc}", name=f"acc{b}_{uc}")
                    for uc in range(UC)
                ]
                for b in range(NB)
            ]

            preloaded = {}

            for wi in range(NW):
                L = WINS[wi]
                last_win = wi == NW - 1
                wt = wwin.pop(wi)
                for q in range(NB):
                    if last_win:
                        if q in preloaded:
                            psums = preloaded.pop(q)
                        else:
                            psums = psum_quarter(q, f"{wi}_{q}")
                            for uc in range(UC):
                                preload_copy(psums[uc], q, uc)
                    else:
                        psums = psum_quarter(q, f"{wi}_{q}")
                    for j in range(L):
                        kt = offs[wi] + j
                        xt = xqpool.tile([P, QB], f16, tag="xq", name=f"x{kt}_{q}")
                        nc.sync.dma_start(
                            xt[:],
                            xt_d[kt * P:(kt + 1) * P, q * QB:(q + 1) * QB],
                        )
                        for uc in range(UC):
                            nc.tensor.matmul(
                                psums[uc][:],
                                wt[:, j * U_SH + uc * P:j * U_SH + (uc + 1) * P],
                                xt[:],
                                start=(j == 0 and not last_win),
                                stop=(j == L - 1),
                                skip_group_check=last_win,
                            )
                    if last_win:
                        # Fused epilogue: tanh(psum + bias) -> fp16,
                        # straight from PSUM on the Activation engine.
                        for uc in range(UC):
                            ot = opool.tile([P, QB], f16, tag="ot", name=f"o{q}_{uc}")
                            nc.scalar.activation(
                                ot[:], psums[uc][:], Tanh,
                                bias=bias_t[uc][:], scale=1.0,
                            )
                            nc.scalar.dma_start(
                                o_d[uc * P:(uc + 1) * P, q * QB:(q + 1) * QB],
                                ot[:],
                            )
                    else:
                        for uc in range(UC):
                            fold(q, uc, psums[uc], wi == 0)
                        if wi == NW - 2 and q >= 2:
                            # Preload the last window's same-parity quarter
                            # right behind these folds.
                            lq = q - 2
                            lps = psum_quarter(lq, f"{NW - 1}_{lq}")
                            for uc in range(UC):
                                preload_copy(lps[uc], lq, uc)
                            preloaded[lq] = lps
                        if q == NB - 1 and wi + W_RES < NW:
                            # Rotate in the next W window tile; its buffer
                            # was freed by this window's last matmul.
                            wwin[wi + W_RES] = w_win_dma(wi + W_RES)

    nc.compile()
    return nc


def _get_nc(key=("v6",)):
    if key not in _NC_CACHE:
        _NC_CACHE[key] = _build_nc()
    return _NC_CACHE[key]


def kernel(x, kernel_vector, bias, nonzero_ind):
    global LAST_RESULT
    from concourse.bass_utils import run_bass_kernel_spmd

    x = np.asarray(x, dtype=np.float32)
    kernel_vector = np.asarray(kernel_vector, dtype=np.float32)
    bias = np.asarray(bias, dtype=np.float32)
    nonzero_ind = np.asarray(nonzero_ind)

    nc = _get_nc()

    # Host scatter: dense fp16 weights, K padded to 157*128.
    rows = nonzero_ind[:, 0].astype(np.int64)
    cols = nonzero_ind[:, 1].astype(np.int64)
    w_full = np.zeros(KPAD * U, np.float32)
    np.add.at(w_full, rows * U + cols, kernel_vector)
    w_full = w_full.reshape(KPAD, U).astype(np.float16)

    # Shared transposed x, fp16, K-padded.
    xt = np.zeros((KPAD, B), np.float16)
    xt[:K] = x.astype(np.float16).T

    in_maps = []
    for c in range(8):
        # Swizzle the W shard partition-major: [128, 157, 512].
        wsh = w_full[:, c * U_SH:(c + 1) * U_SH].reshape(KT, P, U_SH)
        wsh = np.ascontiguousarray(wsh.transpose(1, 0, 2)).reshape(P, KT * U_SH)
        in_maps.append({
            "xt_sh": xt,
            "w_sh": wsh,
            "b_sh": np.ascontiguousarray(
                bias[c * U_SH:(c + 1) * U_SH].reshape(UC, P, 1)
            ),
        })

    kwargs = {}
    if TRACE:
        kwargs = dict(trace=True, trace_cores=list(range(8)))
    res = run_bass_kernel_spmd(nc, in_maps, core_ids=list(range(8)), **kwargs)
    LAST_RESULT = res

    # Device already applied bias + tanh; just assemble (out is [U, B]).
    out_t = np.concatenate(
        [res.results[c]["out_p"] for c in range(8)], axis=0
    )
    return out_t.T.astype(np.float32)


# revision 19
# speedup vs baseline: 1.1210x; 1.0047x over previous
"""Trainium2 kernel for: out = tanh(x @ scatter_nd(nonzero_ind, kernel_vector, (20000, 4096)) + bias).

Strategy (8 NeuronCores):
  - Host builds the dense (20000, 4096) fp16 weight matrix from the COO
    triples, pads K to 157*128 = 20096, pre-transposes x to fp16
    xT (20096, 2048) (shared by all cores), and pre-swizzles each
    core's W shard to partition-major [128, 157, 512] so a whole
    K-window loads as ONE DMA of 128 contiguous 20KB lines.
  - Shard units 8-ways: core c owns output columns [c*512, (c+1)*512).
    No K-split => 157 K-tiles (vs 160 for a padded K-quartering), no
    host-side partial summation.
  - Output-stationary windowed accumulation: K in 8 equal windows
    (7x20 + 17 tiles). Each window runs 4 batch quarter-passes of 512
    columns; a pass accumulates in 4 PSUM banks, ping-ponged by pass
    parity so each pass epilogue has a full pass of slack. Window
    partials fold into SBUF fp32 accumulators (even unit chunks on the
    DVE straight from PSUM; odd ones stage via the Activation engine
    then add on Pool, which cannot touch PSUM).
  - W window tiles: 4 resident in rotation (80 KB/partition); all DMA
    issue is one descriptor-gen per window, so the SP sequencer carries
    only the x stream (sequencer descriptor-gen rate, ~0.6us per
    dma_start, is the scarce resource — per-tile W DMAs starve the PE).
    Window 0 is split into 4 sub-DMAs so the first matmul starts ~9us in.
  - The last window preloads ACC back into PSUM (riding right behind
    window 6's folds) and matmuls continue with start=False, so the
    final epilogue reads PSUM directly: fused out = tanh(psum + bias)
    on the Activation engine, fp16 output.
  - PE warmup: dummy matmuls on a memset tile keep the PE busy through
    the initial DMA fill so its p-state ramp (-> 2.4 GHz) completes
    before real work starts.
  - Host only concatenates/transposes the 8 shards.
"""

import numpy as np

P = 128
B, K, U = 2048, 20000, 4096
KT = 157                 # ceil(20000 / 128)
KPAD = KT * P            # 20096
U_SH = U // 8            # 512 units per core
UC = U_SH // P           # 4 unit chunks per core
QB = 512                 # batch block width (PSUM bank free size)
NB = B // QB             # 4 batch quarters
WINS = [20, 20, 20, 20, 20, 20, 20, 17]   # K-window sizes (sum = 157)
W_RES = 4                # W window tiles resident
N_WARM = 8               # warmup matmuls (free dim 256) covering DMA fill
WARM_F = 256

TRACE = False            # set by test harness for profiled runs
LAST_RESULT = None       # BassKernelResults of the last run (for the harness)

_NC_CACHE = {}


def _build_nc():
    from concourse import bacc
    import concourse.mybir as mybir
    import concourse.tile as tile

    f32 = mybir.dt.float32
    f16 = mybir.dt.float16
    Tanh = mybir.ActivationFunctionType.Tanh
    add_op = mybir.AluOpType.add

    assert sum(WINS) == KT
    NW = len(WINS)
    offs = [sum(WINS[:i]) for i in range(NW)]

    nc = bacc.Bacc("TRN2", target_bir_lowering=False, debug=False)
    xt_d = nc.dram_tensor("xt_sh", [KPAD, B], f16, kind="ExternalInput").ap()
    w_d = nc.dram_tensor("w_sh", [P, KT * U_SH], f16, kind="ExternalInput").ap()
    b_d = nc.dram_tensor("b_sh", [UC, P, 1], f32, kind="ExternalInput").ap()
    o_d = nc.dram_tensor("out_p", [U_SH, B], f16, kind="ExternalOutput").ap()

    with tile.TileContext(nc) as tc:
        with (
            tc.tile_pool(name="wpanel", bufs=1) as wpool,
            tc.tile_pool(name="xquart", bufs=24) as xqpool,
            tc.tile_pool(name="accpool", bufs=1) as apool,
            tc.tile_pool(name="bias", bufs=1) as bpool,
            tc.tile_pool(name="ostage", bufs=4) as opool,
            tc.tile_pool(name="fstage", bufs=4) as spool,
            tc.tile_pool(name="warm", bufs=1) as warmpool,
            tc.tile_pool(name="mpsum", bufs=1, space="PSUM") as mpsum,
        ):
            # ---- epilogue helpers (engine-split; Pool can't touch PSUM) --
            def fold(b, uc, psum, first):
                if uc % 2 == 0:
                    if first:
                        nc.vector.tensor_copy(acc[b][uc][:], psum[:])
                    else:
                        nc.vector.tensor_tensor(
                            acc[b][uc][:], acc[b][uc][:], psum[:], add_op
                        )
                else:
                    if first:
                        nc.scalar.copy(acc[b][uc][:], psum[:])
                    else:
                        st = spool.tile([P, QB], f32, tag="st", name=f"st{b}_{uc}")
                        nc.scalar.copy(st[:], psum[:])
                        nc.gpsimd.tensor_tensor(
                            acc[b][uc][:], acc[b][uc][:], st[:], add_op
                        )

            def preload_copy(psum, b, uc):
                if uc % 2 == 0:
                    nc.vector.tensor_copy(psum[:], acc[b][uc][:])
                else:
                    nc.scalar.copy(psum[:], acc[b][uc][:])

            def psum_quarter(q, label):
                return [
                    mpsum.tile(
                        [P, QB], f32,
                        tag=f"ps{4 * (q % 2) + uc}",
                        name=f"ps_{label}_{uc}",
                    )
                    for uc in range(UC)
                ]

            # PE warmup (parks in bank ps7; real work reaches it late).
            warm = warmpool.tile([P, WARM_F], f16, tag="warm", name="warm")
            nc.vector.memset(warm[:], 0.0)
            warm_ps = mpsum.tile([P, WARM_F], f32, tag="ps7", name="warm_ps")
            for _ in range(N_WARM):
                nc.tensor.matmul(
                    warm_ps[:], warm[:, :P], warm[:], start=True, stop=True
                )

            # W window tiles: one DMA per window (128 lines of L*1KB).
            def w_win_dma(wi, split=1):
                L = WINS[wi]
                t = wpool.tile(
                    [P, L * U_SH], f16, tag=f"w{wi % W_RES}", name=f"ww{wi}"
                )
                c0 = offs[wi] * U_SH
                step = (L + split - 1) // split
                for s in range(0, L, step):
                    n = min(step, L - s)
                    nc.scalar.dma_start(
                        t[:, s * U_SH:(s + n) * U_SH],
                        w_d[:, c0 + s * U_SH:c0 + (s + n) * U_SH],
                    )
                return t

            # Windows 0 (split fine for a fast start) and 1 up front;
            # windows 2/3 are staggered into window 0's pass boundaries so
            # the W burst doesn't crowd out the x stream early on.
            wwin = {}
            wwin[0] = w_win_dma(0, split=8)
            wwin[1] = w_win_dma(1)

            # Per-partition bias chunks (psum partition dim = units).
            bias_t = []
            for uc in range(UC):
                bt = bpool.tile([P, 1], f32, tag=f"b{uc}", name=f"b{uc}")
                nc.scalar.dma_start(bt[:], b_d[uc])
                bias_t.append(bt)

            # SBUF fp32 accumulators, one per (batch quarter, unit chunk).
            acc = [
                [
                    apool.tile([P, QB], f32, tag=f"acc{b}_{uc}", name=f"acc{b}_{uc}")
                    for uc in range(UC)
                ]
                for b in range(NB)
            ]

            preloaded = {}

            for wi in range(NW):
                L = WINS[wi]
                last_win = wi == NW - 1
                wt = wwin.pop(wi)
                for q in range(NB):
                    if last_win:
                        if q in preloaded:
                            psums = preloaded.pop(q)
                        else:
                            psums = psum_quarter(q, f"{wi}_{q}")
                            for uc in range(UC):
                                preload_copy(psums[uc], q, uc)
                    else:
                        psums = psum_quarter(q, f"{wi}_{q}")
                    for j in range(L):
                        kt = offs[wi] + j
                        xt = xqpool.tile([P, QB], f16, tag="xq", name=f"x{kt}_{q}")
                        nc.sync.dma_start(
                            xt[:],
                            xt_d[kt * P:(kt + 1) * P, q * QB:(q + 1) * QB],
                        )
                        for uc in range(UC):
                            nc.tensor.matmul(
                                psums[uc][:],
                                wt[:, j * U_SH + uc * P:j * U_SH + (uc + 1) * P],
                                xt[:],
                                start=(j == 0 and not last_win),
                                stop=(j == L - 1),
                                skip_group_check=last_win,
                            )
                    if last_win:
                        # Fused epilogue: tanh(psum + bias) -> fp16,
                        # straight from PSUM on the Activation engine.
                        for uc in range(UC):
                            ot = opool.tile([P, QB], f16, tag="ot", name=f"o{q}_{uc}")
                            nc.scalar.activation(
                                ot[:], psums[uc][:], Tanh,
                                bias=bias_t[uc][:], scale=1.0,
                            )
                            nc.scalar.dma_start(
                                o_d[uc * P:(uc + 1) * P, q * QB:(q + 1) * QB],
                                ot[:],
                            )
                    else:
                        for uc in range(UC):
                            fold(q, uc, psums[uc], wi == 0)
                        if wi == NW - 2 and q >= 2:
                            # Preload the last window's same-parity quarter
                            # right behind these folds.
                            lq = q - 2
                            lps = psum_quarter(lq, f"{NW - 1}_{lq}")
                            for uc in range(UC):
                                preload_copy(lps[uc], lq, uc)
                            preloaded[lq] = lps
                        if wi == 0 and q == 0:
                            wwin[2] = w_win_dma(2)
                        if wi == 0 and q == 2:
                            wwin[3] = w_win_dma(3)
                        if q == NB - 1 and wi + W_RES < NW:
                            # Rotate in the next W window tile; its buffer
                            # was freed by this window's last matmul.
                            wwin[wi + W_RES] = w_win_dma(wi + W_RES)

    nc.compile()
    return nc


def _get_nc(key=("v6",)):
    if key not in _NC_CACHE:
        _NC_CACHE[key] = _build_nc()
    return _NC_CACHE[key]


def kernel(x, kernel_vector, bias, nonzero_ind):
    global LAST_RESULT
    from concourse.bass_utils import run_bass_kernel_spmd

    x = np.asarray(x, dtype=np.float32)
    kernel_vector = np.asarray(kernel_vector, dtype=np.float32)
    bias = np.asarray(bias, dtype=np.float32)
    nonzero_ind = np.asarray(nonzero_ind)

    nc = _get_nc()

    # Host scatter: dense fp16 weights, K padded to 157*128.
    rows = nonzero_ind[:, 0].astype(np.int64)
    cols = nonzero_ind[:, 1].astype(np.int64)
    w_full = np.zeros(KPAD * U, np.float32)
    np.add.at(w_full, rows * U + cols, kernel_vector)
    w_full = w_full.reshape(KPAD, U).astype(np.float16)

    # Shared transposed x, fp16, K-padded.
    xt = np.zeros((KPAD, B), np.float16)
    xt[:K] = x.astype(np.float16).T

    in_maps = []
    for c in range(8):
        # Swizzle the W shard partition-major: [128, 157, 512].
        wsh = w_full[:, c * U_SH:(c + 1) * U_SH].reshape(KT, P, U_SH)
        wsh = np.ascontiguousarray(wsh.transpose(1, 0, 2)).reshape(P, KT * U_SH)
        in_maps.append({
            "xt_sh": xt,
            "w_sh": wsh,
            "b_sh": np.ascontiguousarray(
                bias[c * U_SH:(c + 1) * U_SH].reshape(UC, P, 1)
            ),
        })

    kwargs = {}
    if TRACE:
        kwargs = dict(trace=True, trace_cores=list(range(8)))
    res = run_bass_kernel_spmd(nc, in_maps, core_ids=list(range(8)), **kwargs)
    LAST_RESULT = res

    # Device already applied bias + tanh; just assemble (out is [U, B]).
    out_t = np.concatenate(
        [res.results[c]["out_p"] for c in range(8)], axis=0
    )
    return out_t.T.astype(np.float32)
